# revision 5
# baseline (speedup 1.0000x reference)
"""Dot-product attention (N=8, TD=1024, TE=2048, D=1024) on 8 trn2 NeuronCores.

Sharding: batch dim N across the 8 cores (fully data-parallel attention).

Per-core kernel strategy:
  - All matmuls contract on the partition dim, so scores are computed as
    S^T[s, t] = sum_d M[s, d] * Q[t, d] with lhsT = mT (d-major) chunks and
    rhs = qT (d-major) chunks.  Host supplies qT/mT (transposed views) so no
    on-chip transposes are needed anywhere.
  - exp is fused on ScalarE: expST = exp(S^T * (1/sqrt(D)) + maskbias) where
    maskbias is a per-partition [128, 1] bias (s is the partition dim), 0 for
    valid s and -1e30 for s >= length.  No max-subtraction is needed: S/32 is
    ~N(0,1) so exp never overflows, and softmax is shift-invariant.
  - Row sums (over s = partitions) via a ones-vector matmul on PE,
    reciprocal on DVE, broadcast over partitions via a K=1 outer-product
    matmul, normalization A^T = expST * recip_bcast on DVE.
  - contexts C[t, d] = sum_s A^T[s, t] * M[s, d]: lhsT = A^T chunks (already
    in the right layout), rhs = natural-layout M chunks.
  - Matmuls run in float32r (fp32 rounded to 11 mantissa bits, TF32-style):
    1 cycle/row on trn2 for moving dim >= 256, i.e. 4x plain-fp32 matmul
    throughput.  DMA-fed operands are pre-rounded on the host; on-chip
    operands (expST, A^T, recip) are produced by engines with float32r
    output dtype so the hardware rounds them.
  - alignments are produced transposed ([TE, TD] per core); the host
    transposes them back during the gather step.
"""

import sys
from contextlib import ExitStack

import numpy as np

sys.path.insert(0, "/opt/trn_rl_repo")

import concourse.bass as bass  # noqa: E402,F401
import concourse.tile as tile  # noqa: E402
from concourse import bacc, mybir  # noqa: E402
from concourse.bass_utils import run_bass_kernel_spmd  # noqa: E402

N, TD, TE, D = 8, 1024, 2048, 1024
SCALE = 1.0 / float(np.sqrt(D))  # 1/32

TB = 256          # t-block size (matmul1 moving dim; >=256 keeps f32r at 1 cyc/row)
NTB = TD // TB    # 4 t-blocks
NSC = TE // 128   # 16 s-chunks of 128 (partition dim of S^T)
NDC = D // 128    # 8 d-chunks of 128 (contraction dim of matmul1)

F32 = mybir.dt.float32
F32R = mybir.dt.float32r

# Set by test.py to capture profiling info; harmless defaults for grading.
TRACE = False
LAST_RESULTS = None


def round_to_f32r(x):
    """Round fp32 array to float32r (11-bit mantissa, RNE), in fp32 bits."""
    u = np.ascontiguousarray(x, dtype=np.float32).view(np.uint32)
    r = (u + 0x7FF + ((u >> 12) & 1)) & 0xFFFFF000
    return r.astype(np.uint32).view(np.float32)


def build_program():
    nc = bacc.Bacc("TRN2", target_bir_lowering=False, debug=False)

    qT = nc.dram_tensor("qT", [D, TD], F32R, kind="ExternalInput").ap()
    mT = nc.dram_tensor("mT", [D, TE], F32R, kind="ExternalInput").ap()
    mN = nc.dram_tensor("mN", [TE, D], F32R, kind="ExternalInput").ap()
    maskb = nc.dram_tensor("maskb", [128, NSC], F32, kind="ExternalInput").ap()
    ones_c = nc.dram_tensor("ones_c", [128, 1], F32R, kind="ExternalInput").ap()
    ones_r = nc.dram_tensor("ones_r", [1, 128], F32R, kind="ExternalInput").ap()

    alignT = nc.dram_tensor("alignT", [TE, TD], F32, kind="ExternalOutput").ap()
    ctx_out = nc.dram_tensor("ctx", [TD, D], F32, kind="ExternalOutput").ap()

    # d-major views: [p, chunk, cols] with p = partition within chunk
    qT_v = qT.rearrange("(dc p) t -> p dc t", p=128)
    mT_v = mT.rearrange("(dc p) s -> p dc s", p=128)
    mN_v = mN.rearrange("(sc p) d -> p sc d", p=128)

    with tile.TileContext(nc) as tc, ExitStack() as ctx:
        consts = ctx.enter_context(tc.tile_pool(name="consts", bufs=1))
        mtp = ctx.enter_context(tc.tile_pool(name="mtp", bufs=1))
        mnp = ctx.enter_context(tc.tile_pool(name="mnp", bufs=1))
        qtp = ctx.enter_context(tc.tile_pool(name="qtp", bufs=2))
        esp = ctx.enter_context(tc.tile_pool(name="esp", bufs=1))
        atp = ctx.enter_context(tc.tile_pool(name="atp", bufs=4))
        csb = ctx.enter_context(tc.tile_pool(name="csb", bufs=2))
        smalls = ctx.enter_context(tc.tile_pool(name="smalls", bufs=2))

        ps_s = ctx.enter_context(tc.tile_pool(name="ps_s", bufs=2, space="PSUM"))
        ps_r = ctx.enter_context(tc.tile_pool(name="ps_r", bufs=1, space="PSUM"))
        ps_b = ctx.enter_context(tc.tile_pool(name="ps_b", bufs=1, space="PSUM"))
        ps_c = ctx.enter_context(tc.tile_pool(name="ps_c", bufs=4, space="PSUM"))

        # --- constants ---
        ones_col = consts.tile([128, 1], F32R)    # lhsT for row-sum matmul
        nc.sync.dma_start(out=ones_col, in_=ones_c)
        ones_row = consts.tile([1, 128], F32R)    # lhsT for partition-broadcast
        nc.sync.dma_start(out=ones_row, in_=ones_r)
        maskb_sb = consts.tile([128, NSC], F32)
        nc.sync.dma_start(out=maskb_sb, in_=maskb)

        # --- resident weights: mT (for scores) and natural M (for contexts) ---
        mt_sb = mtp.tile([128, NDC, TE], F32R)    # 64 KB/partition
        mn_sb = mnp.tile([128, NSC, D], F32R)     # 64 KB/partition
        for sc in range(NSC):
            nc.sync.dma_start(
                out=mt_sb[:, :, sc * 128:(sc + 1) * 128],
                in_=mT_v[:, :, sc * 128:(sc + 1) * 128],
            )
        for sc in range(NSC):
            nc.sync.dma_start(out=mn_sb[:, sc, :], in_=mN_v[:, sc, :])

        for tb in range(NTB):
            tsl = slice(tb * TB, (tb + 1) * TB)

            qt_sb = qtp.tile([128, NDC, TB], F32R)
            nc.sync.dma_start(out=qt_sb, in_=qT_v[:, :, tsl])

            # expST[s, t] for this t-block, all 16 s-chunks
            expst = esp.tile([128, NSC, TB], F32R)

            # --- scores + exp ---
            for sc in range(NSC):
                s_ps = ps_s.tile([128, TB], F32, tag="s_ps")
                for dc in range(NDC):
                    nc.tensor.matmul(
                        s_ps,
                        mt_sb[:, dc, sc * 128:(sc + 1) * 128],
                        qt_sb[:, dc, :],
                        start=(dc == 0),
                        stop=(dc == NDC - 1),
                    )
                nc.scalar.activation(
                    out=expst[:, sc, :],
                    in_=s_ps,
                    func=mybir.ActivationFunctionType.Exp,
                    bias=maskb_sb[:, sc:sc + 1],
                    scale=SCALE,
                )

            # --- row sums over s (partition dim) via ones-matmul ---
            rsum_ps = ps_r.tile([1, TB], F32, tag="rsum")
            for sc in range(NSC):
                nc.tensor.matmul(
                    rsum_ps,
                    ones_col,
                    expst[:, sc, :],
                    start=(sc == 0),
                    stop=(sc == NSC - 1),
                )
            rsum_sb = smalls.tile([1, TB], F32, tag="rsum_sb")
            nc.vector.tensor_copy(rsum_sb, rsum_ps)
            recip_sb = smalls.tile([1, TB], F32, tag="recip_sb")
            nc.vector.reciprocal(recip_sb, rsum_sb)
            recip_r = smalls.tile([1, TB], F32R, tag="recip_r")
            nc.vector.tensor_copy(recip_r, recip_sb)

            # --- broadcast recip across partitions: outer product with ones ---
            rbc_ps = ps_b.tile([128, TB], F32, tag="rbc")
            nc.tensor.matmul(rbc_ps, ones_row, recip_r, start=True, stop=True)

            # --- normalize + store alignments (transposed layout) ---
            at_tiles = []
            for sc in range(NSC):
                at_sb = atp.tile([128, TB], F32R, tag="at")
                nc.vector.tensor_mul(at_sb, expst[:, sc, :], rbc_ps)
                nc.sync.dma_start(
                    out=alignT[sc * 128:(sc + 1) * 128, tsl],
                    in_=at_sb.bitcast(F32),
                )
                at_tiles.append(at_sb)

            # --- contexts: C[t, d] accumulation over s-chunks ---
            c_ps = [
                [
                    ps_c.tile([128, 512], F32, tag="c_ps", name=f"c_ps_{tb}_{i}_{j}")
                    for j in range(2)
                ]
                for i in range(TB // 128)
            ]
            for sc in range(NSC):
                for tsub in range(TB // 128):
                    for dh in range(2):
                        nc.tensor.matmul(
                            c_ps[tsub][dh],
                            at_tiles[sc][:, tsub * 128:(tsub + 1) * 128],
                            mn_sb[:, sc, dh * 512:(dh + 1) * 512],
                            start=(sc == 0),
                            stop=(sc == NSC - 1),
                        )
            for tsub in range(TB // 128):
                c_sb = csb.tile([128, D], F32, tag="c_sb")
                for dh in range(2):
                    nc.scalar.copy(c_sb[:, dh * 512:(dh + 1) * 512], c_ps[tsub][dh])
                t0 = tb * TB + tsub * 128
                nc.sync.dma_start(out=ctx_out[t0:t0 + 128, :], in_=c_sb)

    nc.compile()
    return nc


_NC_CACHE = None


def _get_program():
    global _NC_CACHE
    if _NC_CACHE is None:
        _NC_CACHE = build_program()
    return _NC_CACHE


def make_in_maps(queries, memory, lengths):
    queries = np.asarray(queries, dtype=np.float32)
    memory = np.asarray(memory, dtype=np.float32)
    lengths = np.asarray(lengths, dtype=np.int32)
    # s-index grid matching the maskb [128, NSC] layout: s = sc*128 + p
    s_idx = np.arange(TE, dtype=np.int32).reshape(NSC, 128).T  # [128, NSC]
    ones_c = np.ones((128, 1), dtype=np.float32)
    ones_r = np.ones((1, 128), dtype=np.float32)

    in_maps = []
    for n in range(N):
        in_maps.append(
            {
                "qT": round_to_f32r(queries[n].T),
                "mT": round_to_f32r(memory[n].T),
                "mN": round_to_f32r(memory[n]),
                "maskb": np.where(s_idx < lengths[n], 0.0, -1e30).astype(np.float32),
                "ones_c": ones_c,
                "ones_r": ones_r,
            }
        )
    return in_maps


def kernel(queries, memory, lengths):
    global LAST_RESULTS
    nc = _get_program()
    in_maps = make_in_maps(queries, memory, lengths)

    res = run_bass_kernel_spmd(
        nc, in_maps, core_ids=list(range(N)), trace=TRACE
    )
    LAST_RESULTS = res

    contexts = np.empty((N, TD, D), dtype=np.float32)
    alignments = np.empty((N, TD, TE), dtype=np.float32)
    for n in range(N):
        contexts[n] = res.results[n]["ctx"]
        alignments[n] = res.results[n]["alignT"].T
    return contexts, alignments


# revision 18
# speedup vs baseline: 25.0660x; 25.0660x over previous
"""Dot-product attention (N=8, TD=1024, TE=2048, D=1024) on 8 trn2 NeuronCores.

Sharding: batch dim N across the 8 cores (fully data-parallel attention).

Per-core kernel strategy:
  - All matmuls contract on the partition dim, so scores are computed as
    S^T[s, t] = sum_d M[s, d] * Q[t, d] with lhsT = mT (d-major) chunks and
    rhs = qT (d-major) chunks.  Host supplies qT/mT (transposed views) so no
    on-chip transposes are needed anywhere.
  - exp is fused on ScalarE: expST = exp(S^T * (1/sqrt(D)) + maskbias) where
    maskbias is a per-partition [128, 1] bias (s is the partition dim), 0 for
    valid s and -1e30 for s >= length.  No max-subtraction is needed: S/32 is
    ~N(0,1) so exp never overflows, and softmax is shift-invariant.
  - Row sums (over s = partitions) via a ones-vector matmul on PE,
    reciprocal on DVE, broadcast over partitions via a K=1 outer-product
    matmul, normalization A^T = expST * recip_bcast on DVE.
  - contexts C[t, d] = sum_s A^T[s, t] * M[s, d]: lhsT = A^T chunks (already
    in the right layout), rhs = natural-layout M chunks.
  - Matmuls run in float32r (fp32 rounded to 11 mantissa bits, TF32-style):
    1 cycle/row on trn2 for moving dim >= 256, i.e. 4x plain-fp32 matmul
    throughput.  DMA-fed operands are pre-rounded on the host; on-chip
    operands (expST, A^T, recip) are produced by engines with float32r
    output dtype so the hardware rounds them.
  - alignments are produced transposed ([TE, TD] per core); the host
    transposes them back during the gather step.
"""

import sys
from contextlib import ExitStack

import numpy as np

sys.path.insert(0, "/opt/trn_rl_repo")

import concourse.bass as bass  # noqa: E402,F401
import concourse.tile as tile  # noqa: E402
from concourse import bacc, mybir  # noqa: E402
from concourse.bass_isa import ReduceOp  # noqa: E402
from concourse.bass_utils import run_bass_kernel_spmd  # noqa: E402

N, TD, TE, D = 8, 1024, 2048, 1024
SCALE = 1.0 / float(np.sqrt(D))  # 1/32

TB = 256          # t-block size (matmul1 moving dim; >=256 keeps f32r at 1 cyc/row)
NTB = TD // TB    # 4 t-blocks
NSC = TE // 128   # 16 s-chunks of 128 (partition dim of S^T)
NDC = D // 128    # 8 d-chunks of 128 (contraction dim of matmul1)

F32 = mybir.dt.float32
F32R = mybir.dt.float32r

# Set by test.py to capture profiling info; harmless defaults for grading.
TRACE = False
LAST_RESULTS = None


def round_to_f32r(x):
    """Round fp32 array to float32r (11-bit mantissa, RNE), in fp32 bits."""
    u = np.ascontiguousarray(x, dtype=np.float32).view(np.uint32)
    r = (u + 0x7FF + ((u >> 12) & 1)) & 0xFFFFF000
    return r.astype(np.uint32).view(np.float32)


def build_program(passes=1):
    nc = bacc.Bacc("TRN2", target_bir_lowering=False, debug=False)

    qT = nc.dram_tensor("qT", [D, TD], F32R, kind="ExternalInput").ap()
    mT = nc.dram_tensor("mT", [D, TE], F32R, kind="ExternalInput").ap()
    mN = nc.dram_tensor("mN", [TE, D], F32R, kind="ExternalInput").ap()
    maskb = nc.dram_tensor("maskb", [128, NSC], F32, kind="ExternalInput").ap()

    alignT = nc.dram_tensor("alignT", [TE, TD], F32, kind="ExternalOutput").ap()
    ctx_out = nc.dram_tensor("ctx", [TD, D], F32, kind="ExternalOutput").ap()

    # d-major views: [p, chunk, cols] with p = partition within chunk
    qT_v = qT.rearrange("(dc p) t -> p dc t", p=128)
    mT_v = mT.rearrange("(dc p) s -> p dc s", p=128)
    mN_v = mN.rearrange("(sc p) d -> p sc d", p=128)

    with tile.TileContext(nc) as tc, ExitStack() as ctx:
        consts = ctx.enter_context(tc.tile_pool(name="consts", bufs=1))
        mtp = ctx.enter_context(tc.tile_pool(name="mtp", bufs=1))
        mnp = ctx.enter_context(tc.tile_pool(name="mnp", bufs=1))
        qtp = ctx.enter_context(tc.tile_pool(name="qtp", bufs=2))
        esp = ctx.enter_context(tc.tile_pool(name="esp", bufs=2))
        atp = ctx.enter_context(tc.tile_pool(name="atp", bufs=4))
        csb = ctx.enter_context(tc.tile_pool(name="csb", bufs=2))
        smalls = ctx.enter_context(tc.tile_pool(name="smalls", bufs=2))

        ps_s = ctx.enter_context(tc.tile_pool(name="ps_s", bufs=4, space="PSUM"))
        ps_c = ctx.enter_context(tc.tile_pool(name="ps_c", bufs=4, space="PSUM"))

        # --- constants (gpsimd ring: keeps the sync HWDGE ring free for qt/mt) ---
        maskb_sb = consts.tile([128, NSC], F32)
        nc.gpsimd.dma_start(out=maskb_sb, in_=maskb)

        # --- qt prefetch helper: first t-block's queries load BEFORE the big
        # resident-weight preloads so PE can start within a few us ---
        qt_tiles = {}

        def load_qt(tb):
            qt_sb = qtp.tile(
                [128, NDC, TB], F32R, tag="qt", name=f"qt_sb_{tb}_{len(qt_tiles)}"
            )
            nc.sync.dma_start(out=qt_sb, in_=qT_v[:, :, tb * TB:(tb + 1) * TB])
            qt_tiles[tb] = qt_sb

        load_qt(0)

        # --- resident weights: mT (for scores) and natural M (for contexts).
        # mT preloads on the sync (SP) HWDGE ring.  mN is not needed until the
        # first finish() (~30us in), so its loads are issued from the scalar
        # ring interleaved behind the first t-block's exps — this keeps the
        # full HBM bandwidth on mT while the first score phase races it. ---
        mt_sb = mtp.tile([128, NDC, TE], F32R)    # 64 KB/partition
        mn_sb = mnp.tile([128, NSC, D], F32R)     # 64 KB/partition
        for sc in range(NSC):
            nc.sync.dma_start(
                out=mt_sb[:, :, sc * 128:(sc + 1) * 128],
                in_=mT_v[:, :, sc * 128:(sc + 1) * 128],
            )

        def load_mn(sc):
            nc.scalar.dma_start(out=mn_sb[:, sc, :], in_=mN_v[:, sc, :])

        def emit_scores(tb, uid):
            """mm1 + masked exp for all 16 s-chunks.  Row sums run entirely
            off PE: DVE accumulates the 16 exp chunks elementwise (trailing
            the exps by one chunk), then GPSIMD all-reduces over partitions
            (broadcasting the total to every partition) and DVE reciprocates
            in place — finish() one t-block later multiplies by it."""
            qt_sb = qt_tiles.pop(tb)
            expst = esp.tile([128, NSC, TB], F32R, tag="expst", name=f"expst_{uid}")
            acc = smalls.tile([128, TB], F32, tag="acc", name=f"acc_{uid}")

            for sc in range(NSC):
                s_ps = ps_s.tile([128, TB], F32, tag="s_ps", name=f"s_ps_{uid}_{sc}")
                for dc in range(NDC):
                    nc.tensor.matmul(
                        s_ps,
                        mt_sb[:, dc, sc * 128:(sc + 1) * 128],
                        qt_sb[:, dc, :],
                        start=(dc == 0),
                        stop=(dc == NDC - 1),
                    )
                nc.scalar.activation(
                    out=expst[:, sc, :],
                    in_=s_ps,
                    func=mybir.ActivationFunctionType.Exp,
                    bias=maskb_sb[:, sc:sc + 1],
                    scale=SCALE,
                )
                if uid == 0:
                    load_mn(sc)
                if sc == 0:
                    nc.vector.tensor_copy(acc, expst[:, 0, :])
                else:
                    nc.vector.tensor_add(acc, acc, expst[:, sc, :])

            nc.gpsimd.partition_all_reduce(acc, acc, 128, ReduceOp.add)
            nc.vector.reciprocal(acc, acc)
            return tb, uid, expst, acc

        def emit_finish(state):
            """broadcast recip, normalize + store alignments, context matmuls.
            Runs (in PE order) after the NEXT t-block's score phase, so the
            recip chain above has long completed."""
            tb, uid, expst, recip_bc = state
            tsl = slice(tb * TB, (tb + 1) * TB)

            at_tiles = []
            for sc in range(NSC):
                at_sb = atp.tile([128, TB], F32R, tag="at", name=f"at_sb_{uid}_{sc}")
                nc.vector.tensor_mul(at_sb, expst[:, sc, :], recip_bc)
                nc.scalar.dma_start(
                    out=alignT[sc * 128:(sc + 1) * 128, tsl],
                    in_=at_sb.bitcast(F32),
                )
                at_tiles.append(at_sb)

            c_ps = [
                [
                    ps_c.tile([128, 512], F32, tag="c_ps", name=f"c_ps_{uid}_{i}_{j}")
                    for j in range(2)
                ]
                for i in range(TB // 128)
            ]
            for sc in range(NSC):
                for tsub in range(TB // 128):
                    for dh in range(2):
                        nc.tensor.matmul(
                            c_ps[tsub][dh],
                            at_tiles[sc][:, tsub * 128:(tsub + 1) * 128],
                            mn_sb[:, sc, dh * 512:(dh + 1) * 512],
                            start=(sc == 0),
                            stop=(sc == NSC - 1),
                        )
            for tsub in range(TB // 128):
                c_sb = csb.tile([128, D], F32, tag="c_sb", name=f"c_sb_{uid}_{tsub}")
                for dh in range(2):
                    nc.scalar.copy(c_sb[:, dh * 512:(dh + 1) * 512], c_ps[tsub][dh])
                t0 = tb * TB + tsub * 128
                nc.scalar.dma_start(out=ctx_out[t0:t0 + 128, :], in_=c_sb)

        pass_list = list(range(NTB)) * passes
        pending = None
        for ip, tb in enumerate(pass_list):
            if ip + 1 < len(pass_list):
                load_qt(pass_list[ip + 1])
            state = emit_scores(tb, ip)
            if pending is not None:
                emit_finish(pending)
            pending = state
        emit_finish(pending)

    nc.compile()
    return nc


_NC_CACHE = None


def _get_program():
    global _NC_CACHE
    if _NC_CACHE is None:
        _NC_CACHE = build_program()
    return _NC_CACHE


def make_in_maps(queries, memory, lengths):
    queries = np.asarray(queries, dtype=np.float32)
    memory = np.asarray(memory, dtype=np.float32)
    lengths = np.asarray(lengths, dtype=np.int32)
    # s-index grid matching the maskb [128, NSC] layout: s = sc*128 + p
    s_idx = np.arange(TE, dtype=np.int32).reshape(NSC, 128).T  # [128, NSC]

    in_maps = []
    for n in range(N):
        in_maps.append(
            {
                "qT": round_to_f32r(queries[n].T),
                "mT": round_to_f32r(memory[n].T),
                "mN": round_to_f32r(memory[n]),
                "maskb": np.where(s_idx < lengths[n], 0.0, -1e30).astype(np.float32),
            }
        )
    return in_maps


def kernel(queries, memory, lengths):
    global LAST_RESULTS
    nc = _get_program()
    in_maps = make_in_maps(queries, memory, lengths)

    res = run_bass_kernel_spmd(
        nc, in_maps, core_ids=list(range(N)), trace=TRACE
    )
    LAST_RESULTS = res

    contexts = np.empty((N, TD, D), dtype=np.float32)
    alignments = np.empty((N, TD, TE), dtype=np.float32)
    for n in range(N):
        contexts[n] = res.results[n]["ctx"]
        alignments[n] = res.results[n]["alignT"].T
    return contexts, alignments
